# revision 52
# baseline (speedup 1.0000x reference)
"""Distributed Trainium2 (Bass/Tile) kernel for nn_Attention_10771777978397.

Strategy v2 (tensor-parallel over heads, 8 NeuronCores):
  - Each core computes Q/K projections for its 2 heads DIRECTLY in transposed
    layout [hd, r] via weight-stationary matmuls (lhsT = weight block, rhs = x),
    eliminating all DMA transposes. RoPE is applied in the transposed layout
    with cross-partition-half vector ops (weights column-permuted [evens, odds]
    per head so the pair partner sits 64 partitions away).
  - V is produced row-major [r, hd] (x-stationary) for the AV matmul.
  - Causal attention per (batch, head) in transposed-softmax layout produces
    attnT [o=128, sq] tiles; softmax denominators come from a vector-accumulated
    exp-sum and a single ones-matmul per 512-chunk (instead of one per k-tile).
  - One AllToAll PER BATCH redistributes attnT so core j holds all 16 heads for
    rows j*128..(j+1)*128 of that batch; the wo projection for those rows runs
    overlapped with the next batch's QKV compute. Host concatenates.

All matmuls bf16 with f32 PSUM accumulation. Softmax uses exp without
max-subtraction (logits bounded by construction; masked logits never computed;
causal boundary handled by a multiplicative lower-triangular mask on diagonal
128x128 blocks). Normalization is deferred to the attnT tiles.
"""

import math
import os

import numpy as np
import ml_dtypes

import concourse.bass as bass
import concourse.tile as tile
from concourse.tile import add_dep_helper
from concourse import bacc, mybir
from concourse.bass_utils import run_bass_kernel_spmd

# problem shape (hardcoded per harness contract)
B, S, D, H = 4, 1024, 2048, 16
HD = D // H          # 128
NCORES = 8
HPC = H // NCORES    # 2 heads per core
OL = HPC * HD        # 256 local o-dim
R = B * S            # 4096 rows
SCALE = 1.0 / math.sqrt(HD)
NWARM = 52           # PE warm-up matmuls (HAM clock-gate bridge)

BF16 = mybir.dt.bfloat16
F32 = mybir.dt.float32
NPBF16 = ml_dtypes.bfloat16
Copy = mybir.ActivationFunctionType.Copy
Exp = mybir.ActivationFunctionType.Exp

_CACHED = {}


def _build():
    nc = bacc.Bacc("TRN2", target_bir_lowering=False, debug=False,
                   num_devices=NCORES, name="attn_tp2")

    xt = nc.declare_dram_parameter("xt", [D, R], BF16, isOutput=False)
    wqkt = nc.declare_dram_parameter("wqkt", [D, 2 * OL], BF16, isOutput=False)
    wvt = nc.declare_dram_parameter("wvt", [D, OL], BF16, isOutput=False)
    wot = nc.declare_dram_parameter("wot", [D, D], BF16, isOutput=False)
    ctab = nc.declare_dram_parameter("ctab", [128, S], BF16, isOutput=False)
    stab = nc.declare_dram_parameter("stab", [128, S], BF16, isOutput=False)
    tri = nc.declare_dram_parameter("tri", [128, 128], BF16, isOutput=False)
    out = nc.declare_dram_parameter("out", [B, 128, D], F32, isOutput=True)

    xt_v = xt.ap().rearrange("(k p) r -> p k r", p=128)      # [128,16,R]
    wqk_v = wqkt.ap().rearrange("(k p) o -> p k o", p=128)   # [128,16,512]
    wv_v = wvt.ap().rearrange("(k p) o -> p k o", p=128)     # [128,16,256]
    wo_v = wot.ap().rearrange("(k p) o -> p k o", p=128)     # [128,16,D]

    with tile.TileContext(nc) as tc:
        with (
            tc.tile_pool(name="persist", bufs=1) as persist,
            tc.tile_pool(name="xtp", bufs=2) as xtp,
            tc.tile_pool(name="qktp", bufs=2) as qktp,
            tc.tile_pool(name="ropep", bufs=2) as ropep,
            tc.tile_pool(name="expp", bufs=4) as expp,
            tc.tile_pool(name="attp", bufs=3) as attp,
            tc.tile_pool(name="normp", bufs=2) as normp,
            tc.tile_pool(name="rtp", bufs=2) as rtp,
            tc.tile_pool(name="fop", bufs=2) as fop,
            tc.tile_pool(name="bigps", bufs=3, space="PSUM") as bigps,
            tc.tile_pool(name="scps", bufs=2, space="PSUM") as scps,
            tc.tile_pool(name="pops", bufs=2, space="PSUM") as pops,
            tc.tile_pool(name="t2ps", bufs=1, space="PSUM") as t2ps,
            tc.tile_pool(name="dram", bufs=1, space="DRAM") as dram,
        ):
            # ---- persistent SBUF loads --------------------------------------
            # wqk o-group-major (host pre-ordered by emission order of the
            # four QK groups) so the first group's weights land in ~1.5us;
            # rope tables + wv go on the gpsimd queue to keep sync free
            wqk_sb = persist.tile([128, 4, 16, 128], BF16)
            wv_sb = persist.tile([128, 16, OL], BF16)
            wqk_g = wqkt.ap().rearrange("(k p) (g o) -> p g k o", p=128, g=4)
            for grp in (0, 2, 1, 3):   # QK-group emission order
                # first group in fine chunks so granule-0 matmuls start on
                # k=0 while the rest of the startup load is still in flight
                step = 4 if grp == 0 else 16
                for kc in range(0, 16, step):
                    nc.sync.dma_start(out=wqk_sb[:, grp, kc:kc + step, :],
                                      in_=wqk_g[:, grp, kc:kc + step, :])
            ctab_sb = persist.tile([128, S], BF16)
            stab_sb = persist.tile([128, S], BF16)
            nc.gpsimd.dma_start(out=ctab_sb[:], in_=ctab.ap())
            nc.gpsimd.dma_start(out=stab_sb[:], in_=stab.ap())
            for kc in range(0, 16, 4):
                nc.gpsimd.dma_start(out=wv_sb[:, kc:kc + 4, :],
                                    in_=wv_v[:, kc:kc + 4, :])
            tri_sb = persist.tile([128, 128], BF16)
            nc.sync.dma_start(out=tri_sb[:], in_=tri.ap())
            ones_sb = persist.tile([128, 1], BF16)
            nc.vector.memset(ones_sb[:], 1.0)

            # first x granule (scalar HWDGE queue, ahead of everything else)
            xg_tiles = {}
            xg_dmas = {}

            def prefetch_xg(g, step=8):
                xg = xtp.tile([128, 16, 512], BF16, tag="xg", name=f"xg{g}")
                dmas = []
                for kc in range(0, 16, step):
                    dmas.append(nc.scalar.dma_start(
                        out=xg[:, kc:kc + step, :],
                        in_=xt_v[:, kc:kc + step, g * 512:(g + 1) * 512]))
                xg_tiles[g] = xg
                xg_dmas[g] = dmas
                return dmas

            prefetch_xg(0, step=2)

            wo_sb = persist.tile([128, 16, D], BF16)

            # PE pre-warm: dependency-free matmuls bridge the HAM clock gate
            # until the first real matmuls are ready
            warm_sb = persist.tile([128, 512], BF16, name="warm_sb")
            nc.vector.memset(warm_sb[:], 0.0)
            for w in range(NWARM):
                w_ps = scps.tile([128, 512], F32, tag="sc", name=f"warm{w}")
                nc.tensor.matmul(out=w_ps[:], lhsT=warm_sb[:, :128],
                                 rhs=warm_sb[:], start=True, stop=True)

            send = [dram.tile([NCORES, OL, 128], BF16, name=f"send{b}",
                              tag=f"send{b}") for b in range(B - 1)]
            recv = [dram.tile([NCORES, OL, 128], BF16, name=f"recv{b}",
                              tag=f"recv{b}") for b in range(B - 1)]
            # last batch: per-head buffers so its two AllToAlls pipeline
            sendh = [dram.tile([NCORES, 128, 128], BF16, name=f"sendh{h}",
                               tag=f"sendh{h}") for h in range(HPC)]
            recvh = [dram.tile([NCORES, 128, 128], BF16, name=f"recvh{h}",
                               tag=f"recvh{h}") for h in range(HPC)]
            qkt_tiles = {}
            vsb_tiles = {}

            # ---- phase blocks ----------------------------------------------
            def emit_granule(b, gi):
                g = 2 * b + gi
                if g + 1 < 2 * B:
                    prefetch_xg(g + 1)
                xg = xg_tiles.pop(g)
                soff = gi * 512
                if gi == 0:
                    QT = qktp.tile([128, HPC, 8, 128], BF16, tag="qt",
                                   name=f"qt{b}")
                    KT = qktp.tile([128, HPC, 8, 128], BF16, tag="kt",
                                   name=f"kt{b}")
                    Vsb = qktp.tile([128, 8, OL], BF16, tag="v", name=f"v{b}")
                    qkt_tiles[b] = (QT, KT)
                    vsb_tiles[b] = Vsb
                QT, KT = qkt_tiles[b]
                Vsb = vsb_tiles[b]

                def qk_group(h, qk):
                    ps = bigps.tile([128, 512], F32, tag="big",
                                    name=f"qkps{g}_{h}_{qk}")
                    grp = qk * 2 + h
                    for k in range(16):
                        nc.tensor.matmul(out=ps[:], lhsT=wqk_sb[:, grp, k, :],
                                         rhs=xg[:, k, :], start=(k == 0),
                                         stop=(k == 15))
                    # t2 lives in PSUM: a TensorTensor op may mix partition
                    # bases only when one input is PSUM (SB+SB must match)
                    t1 = ropep.tile([128, 512], BF16, tag="t1", name=f"t1_{g}{h}{qk}")
                    t2 = t2ps.tile([128, 512], F32, tag="t2", name=f"t2_{g}{h}{qk}")
                    nc.vector.tensor_mul(t1[:], ps[:], ctab_sb[:, soff:soff + 512])
                    nc.vector.tensor_mul(t2[:], ps[:], stab_sb[:, soff:soff + 512])
                    dst = QT if qk == 0 else KT
                    lo = dst[0:64, h, gi * 4:gi * 4 + 4, :]
                    hi = dst[64:128, h, gi * 4:gi * 4 + 4, :]
                    nc.vector.tensor_sub(lo, t1[0:64, :], t2[64:128, :])
                    nc.vector.tensor_add(hi, t2[0:64, :], t1[64:128, :])

                last_scalar = [None]

                def v_group(pair):
                    vp = bigps.tile([128, 512], F32, tag="big",
                                    name=f"vps{g}_{pair}")
                    for u2 in range(2):
                        u = pair * 2 + u2
                        for k in range(16):
                            nc.tensor.matmul(
                                out=vp[:, u2 * OL:(u2 + 1) * OL],
                                lhsT=xg[:, k, u * 128:(u + 1) * 128],
                                rhs=wv_sb[:, k, :], start=(k == 0), stop=(k == 15))
                        last_scalar[0] = nc.scalar.activation(
                            out=Vsb[:, gi * 4 + u, :],
                            in_=vp[:, u2 * OL:(u2 + 1) * OL], func=Copy)

                qk_group(0, 0)
                v_group(0)
                qk_group(0, 1)
                v_group(1)
                qk_group(1, 0)
                qk_group(1, 1)
                return last_scalar[0]

            def emit_attention(b, h):
                QT, KT = qkt_tiles[b]
                Vsb = vsb_tiles[b]
                last_exp = None
                for c in range(2):              # sq chunks of 512
                    o_ps = pops.tile([128, 512], F32, tag="po",
                                     name=f"po{b}_{h}_{c}")
                    exs = expp.tile([128, 512], BF16, tag="exs",
                                    name=f"exs{b}_{h}_{c}", bufs=2)
                    njt = 4 * c + 4             # sk tiles for this chunk
                    for j in range(njt):
                        col0 = max(0, (j - 4 * c) * 128)
                        t0 = 4 * c
                        s_ps = scps.tile([128, 512], F32, tag="sc",
                                         name=f"sc{b}_{h}_{c}_{j}")
                        nc.tensor.matmul(
                            out=s_ps[:, col0:], lhsT=KT[:, h, j, :],
                            rhs=QT[:, h, t0 + col0 // 128:t0 + 4, :],
                            start=True, stop=True)
                        ex = expp.tile([128, 512], BF16, tag="ex",
                                       name=f"ex{b}_{h}_{c}_{j}")
                        last_exp = nc.scalar.activation(
                            out=ex[:, col0:], in_=s_ps[:, col0:],
                            func=Exp, scale=SCALE)
                        if j - 4 * c >= 0:      # diagonal block: causal mask
                            nc.vector.tensor_mul(
                                ex[:, col0:col0 + 128], ex[:, col0:col0 + 128],
                                tri_sb[:])
                        if j == 0:
                            nc.vector.tensor_copy(out=exs[:], in_=ex[:])
                        else:
                            nc.vector.tensor_add(exs[:, col0:], exs[:, col0:],
                                                 ex[:, col0:])
                        nc.tensor.matmul(out=o_ps[:, col0:],
                                         lhsT=Vsb[:, j, h * 128:(h + 1) * 128],
                                         rhs=ex[:, col0:],
                                         start=(j == 0), stop=(j == njt - 1))
                    cs_ps = scps.tile([1, 512], F32, tag="sc",
                                      name=f"cs{b}_{h}_{c}")
                    nc.tensor.matmul(out=cs_ps[:], lhsT=ones_sb[:], rhs=exs[:],
                                     start=True, stop=True)
                    rcp = normp.tile([1, 512], F32, tag="rcp",
                                     name=f"rcp{b}_{h}_{c}")
                    nc.vector.reciprocal_approx_fast(out=rcp[:], in_=cs_ps[:])
                    bc = normp.tile([128, 512], F32, tag="bc",
                                    name=f"bc{b}_{h}_{c}")
                    nc.gpsimd.partition_broadcast(bc[:], rcp[:])
                    att = attp.tile([128, 512], BF16, tag="att",
                                    name=f"att{b}_{h}_{c}")
                    nc.vector.tensor_mul(att[:], o_ps[:], bc[:])
                    # one DMA: att [o=128, (jj r)] -> send slices [jj, o, r]
                    if b == B - 1:
                        dst = sendh[h][c * 4:(c + 1) * 4, :, :] \
                            .rearrange("a o r -> o a r")
                    else:
                        dst = send[b][c * 4:(c + 1) * 4,
                                      h * 128:(h + 1) * 128, :] \
                            .rearrange("a o r -> o a r")
                    nc.sync.dma_start(out=dst, in_=att[:])
                return last_exp

            def emit_wo(b, order_after=None):
                # rT in halves so the first wo matmuls start before the whole
                # receive buffer has landed. order_after keeps these loads
                # BEHIND earlier scalar-queue work: they wait on the AllToAll
                # and would otherwise head-of-line-block the exp activations.
                rT = rtp.tile([128, 16, 128], BF16, tag="rt", name=f"rt{b}")
                rv = recv[b][:].rearrange("c (hh p) r -> p (c hh) r", hh=2)
                d1 = nc.scalar.dma_start(out=rT[:, 0:8, :], in_=rv[:, 0:8, :])
                d2 = nc.scalar.dma_start(out=rT[:, 8:16, :], in_=rv[:, 8:16, :])
                if order_after is not None:
                    for dd in (d1, d2):
                        add_dep_helper(dd.ins, order_after.ins, sync=False,
                                       reason="rT load after scalar-queue work")
                for dc in range(4):
                    f_ps = bigps.tile([128, 512], F32, tag="big",
                                      name=f"fps{b}_{dc}")
                    for m in range(16):
                        nc.tensor.matmul(
                            out=f_ps[:], lhsT=rT[:, m, :],
                            rhs=wo_sb[:, m, dc * 512:(dc + 1) * 512],
                            start=(m == 0), stop=(m == 15))
                    fo = fop.tile([128, 512], F32, tag="fo", name=f"fo{b}_{dc}")
                    nc.scalar.activation(out=fo[:], in_=f_ps[:], func=Copy)
                    nc.sync.dma_start(
                        out=out.ap()[b, :, dc * 512:(dc + 1) * 512], in_=fo[:])

            def emit_wo_tail(b, order_after=None):
                rTs = []
                for h in range(HPC):
                    rT = rtp.tile([128, 8, 128], BF16, tag="rt", bufs=2,
                                  name=f"rtt{h}")
                    dd = nc.scalar.dma_start(
                        out=rT[:], in_=recvh[h][:].rearrange("c p r -> p c r"))
                    if order_after is not None:
                        add_dep_helper(dd.ins, order_after.ins, sync=False,
                                       reason="tail rT after last exp")
                    rTs.append(rT)
                # all h0 half-contractions first: they only need the first
                # AllToAll, so the PE starts ~15us before the h1 data lands
                fps = [bigps.tile([128, 512], F32, tag="big", name=f"fpt{dc}")
                       if dc < 2 else
                       pops.tile([128, 512], F32, tag="po", name=f"fpt{dc}")
                       for dc in range(4)]
                for h in range(HPC):
                    for dc in range(4):
                        for cc in range(NCORES):
                            nc.tensor.matmul(
                                out=fps[dc][:], lhsT=rTs[h][:, cc, :],
                                rhs=wo_sb[:, 2 * cc + h, dc * 512:(dc + 1) * 512],
                                start=(h == 0 and cc == 0),
                                stop=(h == 1 and cc == NCORES - 1))
                        if h == 1:
                            fo = fop.tile([128, 512], F32, tag="fo",
                                          name=f"fot{dc}")
                            nc.scalar.activation(out=fo[:], in_=fps[dc][:],
                                                 func=Copy)
                            nc.sync.dma_start(
                                out=out.ap()[b, :, dc * 512:(dc + 1) * 512],
                                in_=fo[:])

            # ---- schedule ---------------------------------------------------
            for b in range(B):
                g_last = emit_granule(b, 0)
                if b == 1:
                    # wo weights: gated behind the batch-1 x loads so they
                    # don't steal HBM bandwidth from the startup pipeline
                    for dc in range(4):
                        d = nc.gpsimd.dma_start(
                            out=wo_sb[:, :, dc * 512:(dc + 1) * 512],
                            in_=wo_v[:, :, dc * 512:(dc + 1) * 512])
                        add_dep_helper(d.ins, xg_dmas[2][-1].ins, sync=True,
                                       reason="wo load after startup loads")
                if b >= 2:
                    emit_wo(b - 2, order_after=g_last)
                emit_granule(b, 1)
                emit_attention(b, 0)
                if b == B - 1:
                    nc.gpsimd.collective_compute(
                        "AllToAll", mybir.AluOpType.bypass,
                        replica_groups=[list(range(NCORES))],
                        ins=[sendh[0].opt()], outs=[recvh[0].opt()])
                last_exp = emit_attention(b, 1)
                if b == B - 1:
                    nc.gpsimd.collective_compute(
                        "AllToAll", mybir.AluOpType.bypass,
                        replica_groups=[list(range(NCORES))],
                        ins=[sendh[1].opt()], outs=[recvh[1].opt()])
                else:
                    nc.gpsimd.collective_compute(
                        "AllToAll", mybir.AluOpType.bypass,
                        replica_groups=[list(range(NCORES))],
                        ins=[send[b].opt()], outs=[recv[b].opt()])
            # wo for batch B-2 lands here: its 17us of ready PE work covers
            # the last batch's AllToAll latency before the tail contraction
            emit_wo(B - 2, order_after=last_exp)
            emit_wo_tail(B - 1, order_after=last_exp)

    nc.compile()
    return nc


def _prep_inputs(x, freqs, wq, wk, wv, wo):
    x = np.asarray(x, np.float32)
    freqs = np.asarray(freqs, np.float32)
    wq = np.asarray(wq, np.float32)
    wk = np.asarray(wk, np.float32)
    wv = np.asarray(wv, np.float32)
    wo = np.asarray(wo, np.float32)

    xt = np.ascontiguousarray(x.reshape(R, D).T).astype(NPBF16)
    wot = np.ascontiguousarray(wo.T).astype(NPBF16)

    cos = np.cos(freqs).T    # [64, S]
    sin = np.sin(freqs).T
    ctab = np.concatenate([cos, cos], axis=0).astype(NPBF16)   # [128, S]
    stab = np.concatenate([sin, sin], axis=0).astype(NPBF16)

    tri = np.tril(np.ones((128, 128), np.float32)).T.copy()  # tri[p,f]=1 if p<=f
    tri = tri.astype(NPBF16)

    in_maps = []
    for core in range(NCORES):
        cols = []
        for hh in range(HPC):
            head = core * HPC + hh
            rows = np.arange(head * HD, (head + 1) * HD)
            cols.append(np.concatenate([rows[0::2], rows[1::2]]))
        cols = np.concatenate(cols)
        vcols = np.arange(core * OL, (core + 1) * OL)
        wqk_host = np.concatenate([wq[cols, :].T, wk[cols, :].T], axis=1)
        in_maps.append({
            "xt": xt,
            "wqkt": np.ascontiguousarray(wqk_host).astype(NPBF16),
            "wvt": np.ascontiguousarray(wv[vcols, :].T).astype(NPBF16),
            "wot": wot,
            "ctab": ctab,
            "stab": stab,
            "tri": tri,
        })
    return in_maps


def kernel(x, freqs, mask, wq, wk, wv, wo, start_pos, _trace=False):
    # mask is the standard causal mask (applied structurally on-device);
    # start_pos is 0 for this problem shape.
    if "nc" not in _CACHED:
        _CACHED["nc"] = _build()
    nc = _CACHED["nc"]
    in_maps = _prep_inputs(x, freqs, wq, wk, wv, wo)
    # warmup execution settles PJRT dispatch, NRT comm init, core-start skew
    if os.environ.get("ATTN_TP_WARMUP", "1") == "1" and "warm" not in _CACHED:
        run_bass_kernel_spmd(nc, in_maps, core_ids=list(range(NCORES)), trace=False)
        _CACHED["warm"] = True
    res = run_bass_kernel_spmd(nc, in_maps, core_ids=list(range(NCORES)),
                               trace=_trace)
    kernel.last_results = res
    # res[j]["out"]: [B, 128, D] = rows j*128..(j+1)*128 of each batch
    parts = np.stack([res.results[j]["out"] for j in range(NCORES)], axis=1)
    return np.ascontiguousarray(parts.reshape(B, S, D)).astype(np.float32)


# revision 54
# speedup vs baseline: 1.0096x; 1.0096x over previous
"""Distributed Trainium2 (Bass/Tile) kernel for nn_Attention_10771777978397.

Strategy v2 (tensor-parallel over heads, 8 NeuronCores):
  - Each core computes Q/K projections for its 2 heads DIRECTLY in transposed
    layout [hd, r] via weight-stationary matmuls (lhsT = weight block, rhs = x),
    eliminating all DMA transposes. RoPE is applied in the transposed layout
    with cross-partition-half vector ops (weights column-permuted [evens, odds]
    per head so the pair partner sits 64 partitions away).
  - V is produced row-major [r, hd] (x-stationary) for the AV matmul.
  - Causal attention per (batch, head) in transposed-softmax layout produces
    attnT [o=128, sq] tiles; softmax denominators come from a vector-accumulated
    exp-sum and a single ones-matmul per 512-chunk (instead of one per k-tile).
  - One AllToAll PER BATCH redistributes attnT so core j holds all 16 heads for
    rows j*128..(j+1)*128 of that batch; the wo projection for those rows runs
    overlapped with the next batch's QKV compute. Host concatenates.

All matmuls bf16 with f32 PSUM accumulation. Softmax uses exp without
max-subtraction (logits bounded by construction; masked logits never computed;
causal boundary handled by a multiplicative lower-triangular mask on diagonal
128x128 blocks). Normalization is deferred to the attnT tiles.
"""

import math
import os

import numpy as np
import ml_dtypes

import concourse.bass as bass
import concourse.tile as tile
from concourse.tile import add_dep_helper
from concourse import bacc, mybir
from concourse.bass_utils import run_bass_kernel_spmd

# problem shape (hardcoded per harness contract)
B, S, D, H = 4, 1024, 2048, 16
HD = D // H          # 128
NCORES = 8
HPC = H // NCORES    # 2 heads per core
OL = HPC * HD        # 256 local o-dim
R = B * S            # 4096 rows
SCALE = 1.0 / math.sqrt(HD)
NWARM = 52           # PE warm-up matmuls (HAM clock-gate bridge)

BF16 = mybir.dt.bfloat16
F32 = mybir.dt.float32
NPBF16 = ml_dtypes.bfloat16
Copy = mybir.ActivationFunctionType.Copy
Exp = mybir.ActivationFunctionType.Exp

_CACHED = {}


def _build():
    nc = bacc.Bacc("TRN2", target_bir_lowering=False, debug=False,
                   num_devices=NCORES, name="attn_tp2")

    xt = nc.declare_dram_parameter("xt", [D, R], BF16, isOutput=False)
    wqkt = nc.declare_dram_parameter("wqkt", [D, 2 * OL], BF16, isOutput=False)
    wvt = nc.declare_dram_parameter("wvt", [D, OL], BF16, isOutput=False)
    wot = nc.declare_dram_parameter("wot", [D, D], BF16, isOutput=False)
    ctab = nc.declare_dram_parameter("ctab", [128, S], BF16, isOutput=False)
    stab = nc.declare_dram_parameter("stab", [128, S], BF16, isOutput=False)
    tri = nc.declare_dram_parameter("tri", [128, 128], BF16, isOutput=False)
    out = nc.declare_dram_parameter("out", [B, 128, D], F32, isOutput=True)

    xt_v = xt.ap().rearrange("(k p) r -> p k r", p=128)      # [128,16,R]
    wqk_v = wqkt.ap().rearrange("(k p) o -> p k o", p=128)   # [128,16,512]
    wv_v = wvt.ap().rearrange("(k p) o -> p k o", p=128)     # [128,16,256]
    wo_v = wot.ap().rearrange("(k p) o -> p k o", p=128)     # [128,16,D]

    with tile.TileContext(nc) as tc:
        with (
            tc.tile_pool(name="persist", bufs=1) as persist,
            tc.tile_pool(name="xtp", bufs=2) as xtp,
            tc.tile_pool(name="qktp", bufs=2) as qktp,
            tc.tile_pool(name="ropep", bufs=2) as ropep,
            tc.tile_pool(name="expp", bufs=4) as expp,
            tc.tile_pool(name="attp", bufs=3) as attp,
            tc.tile_pool(name="normp", bufs=2) as normp,
            tc.tile_pool(name="rtp", bufs=2) as rtp,
            tc.tile_pool(name="fop", bufs=2) as fop,
            tc.tile_pool(name="bigps", bufs=3, space="PSUM") as bigps,
            tc.tile_pool(name="scps", bufs=2, space="PSUM") as scps,
            tc.tile_pool(name="pops", bufs=2, space="PSUM") as pops,
            tc.tile_pool(name="t2ps", bufs=1, space="PSUM") as t2ps,
            tc.tile_pool(name="dram", bufs=1, space="DRAM") as dram,
        ):
            # ---- persistent SBUF loads --------------------------------------
            # wqk o-group-major (host pre-ordered by emission order of the
            # four QK groups) so the first group's weights land in ~1.5us;
            # rope tables + wv go on the gpsimd queue to keep sync free
            wqk_sb = persist.tile([128, 4, 16, 128], BF16)
            wv_sb = persist.tile([128, 16, OL], BF16)
            wqk_g = wqkt.ap().rearrange("(k p) (g o) -> p g k o", p=128, g=4)
            for grp in (0, 2, 1, 3):   # QK-group emission order
                # first group in fine chunks so granule-0 matmuls start on
                # k=0 while the rest of the startup load is still in flight
                step = 4 if grp == 0 else 16
                for kc in range(0, 16, step):
                    nc.sync.dma_start(out=wqk_sb[:, grp, kc:kc + step, :],
                                      in_=wqk_g[:, grp, kc:kc + step, :])
            ctab_sb = persist.tile([128, S], BF16)
            stab_sb = persist.tile([128, S], BF16)
            nc.gpsimd.dma_start(out=ctab_sb[:], in_=ctab.ap())
            nc.gpsimd.dma_start(out=stab_sb[:], in_=stab.ap())
            for kc in range(0, 16, 4):
                nc.gpsimd.dma_start(out=wv_sb[:, kc:kc + 4, :],
                                    in_=wv_v[:, kc:kc + 4, :])
            tri_sb = persist.tile([128, 128], BF16)
            nc.sync.dma_start(out=tri_sb[:], in_=tri.ap())
            ones_sb = persist.tile([128, 1], BF16)
            nc.vector.memset(ones_sb[:], 1.0)

            # first x granule (scalar HWDGE queue, ahead of everything else)
            xg_tiles = {}
            xg_dmas = {}

            def prefetch_xg(g, step=8):
                xg = xtp.tile([128, 16, 512], BF16, tag="xg", name=f"xg{g}")
                dmas = []
                for kc in range(0, 16, step):
                    dmas.append(nc.scalar.dma_start(
                        out=xg[:, kc:kc + step, :],
                        in_=xt_v[:, kc:kc + step, g * 512:(g + 1) * 512]))
                xg_tiles[g] = xg
                xg_dmas[g] = dmas
                return dmas

            prefetch_xg(0, step=2)

            wo_sb = persist.tile([128, 16, D], BF16)

            # PE pre-warm: dependency-free matmuls bridge the HAM clock gate
            # until the first real matmuls are ready
            warm_sb = persist.tile([128, 512], BF16, name="warm_sb")
            nc.vector.memset(warm_sb[:], 0.0)
            for w in range(NWARM):
                w_ps = scps.tile([128, 512], F32, tag="sc", name=f"warm{w}")
                nc.tensor.matmul(out=w_ps[:], lhsT=warm_sb[:, :128],
                                 rhs=warm_sb[:], start=True, stop=True)

            send = [dram.tile([NCORES, OL, 128], BF16, name=f"send{b}",
                              tag=f"send{b}") for b in range(B - 1)]
            recv = [dram.tile([NCORES, OL, 128], BF16, name=f"recv{b}",
                              tag=f"recv{b}") for b in range(B - 1)]
            # last batch: per-head buffers so its two AllToAlls pipeline
            sendh = [dram.tile([NCORES, 128, 128], BF16, name=f"sendh{h}",
                               tag=f"sendh{h}") for h in range(HPC)]
            recvh = [dram.tile([NCORES, 128, 128], BF16, name=f"recvh{h}",
                               tag=f"recvh{h}") for h in range(HPC)]
            qkt_tiles = {}
            vsb_tiles = {}

            # ---- phase blocks ----------------------------------------------
            def emit_granule(b, gi):
                g = 2 * b + gi
                if g + 1 < 2 * B:
                    prefetch_xg(g + 1)
                xg = xg_tiles.pop(g)
                soff = gi * 512
                if gi == 0:
                    QT = qktp.tile([128, HPC, 8, 128], BF16, tag="qt",
                                   name=f"qt{b}")
                    KT = qktp.tile([128, HPC, 8, 128], BF16, tag="kt",
                                   name=f"kt{b}")
                    Vsb = qktp.tile([128, 8, OL], BF16, tag="v", name=f"v{b}")
                    qkt_tiles[b] = (QT, KT)
                    vsb_tiles[b] = Vsb
                QT, KT = qkt_tiles[b]
                Vsb = vsb_tiles[b]

                def qk_group(h, qk):
                    ps = bigps.tile([128, 512], F32, tag="big",
                                    name=f"qkps{g}_{h}_{qk}")
                    grp = qk * 2 + h
                    for k in range(16):
                        nc.tensor.matmul(out=ps[:], lhsT=wqk_sb[:, grp, k, :],
                                         rhs=xg[:, k, :], start=(k == 0),
                                         stop=(k == 15))
                    # t2 lives in PSUM: a TensorTensor op may mix partition
                    # bases only when one input is PSUM (SB+SB must match)
                    t1 = ropep.tile([128, 512], BF16, tag="t1", name=f"t1_{g}{h}{qk}")
                    t2 = t2ps.tile([128, 512], F32, tag="t2", name=f"t2_{g}{h}{qk}")
                    nc.vector.tensor_mul(t1[:], ps[:], ctab_sb[:, soff:soff + 512])
                    nc.vector.tensor_mul(t2[:], ps[:], stab_sb[:, soff:soff + 512])
                    dst = QT if qk == 0 else KT
                    lo = dst[0:64, h, gi * 4:gi * 4 + 4, :]
                    hi = dst[64:128, h, gi * 4:gi * 4 + 4, :]
                    nc.vector.tensor_sub(lo, t1[0:64, :], t2[64:128, :])
                    nc.vector.tensor_add(hi, t2[0:64, :], t1[64:128, :])

                last_scalar = [None]

                def v_group(pair):
                    vp = bigps.tile([128, 512], F32, tag="big",
                                    name=f"vps{g}_{pair}")
                    for u2 in range(2):
                        u = pair * 2 + u2
                        for k in range(16):
                            nc.tensor.matmul(
                                out=vp[:, u2 * OL:(u2 + 1) * OL],
                                lhsT=xg[:, k, u * 128:(u + 1) * 128],
                                rhs=wv_sb[:, k, :], start=(k == 0), stop=(k == 15))
                        last_scalar[0] = nc.scalar.activation(
                            out=Vsb[:, gi * 4 + u, :],
                            in_=vp[:, u2 * OL:(u2 + 1) * OL], func=Copy)

                qk_group(0, 0)
                v_group(0)
                qk_group(0, 1)
                v_group(1)
                qk_group(1, 0)
                qk_group(1, 1)
                return last_scalar[0]

            def emit_attention(b, h, chunks=(0, 1)):
                QT, KT = qkt_tiles[b]
                Vsb = vsb_tiles[b]
                last_exp = None
                for c in chunks:                # sq chunks of 512
                    o_ps = pops.tile([128, 512], F32, tag="po",
                                     name=f"po{b}_{h}_{c}")
                    exs = expp.tile([128, 512], BF16, tag="exs",
                                    name=f"exs{b}_{h}_{c}", bufs=2)
                    njt = 4 * c + 4             # sk tiles for this chunk
                    for j in range(njt):
                        col0 = max(0, (j - 4 * c) * 128)
                        t0 = 4 * c
                        s_ps = scps.tile([128, 512], F32, tag="sc",
                                         name=f"sc{b}_{h}_{c}_{j}")
                        nc.tensor.matmul(
                            out=s_ps[:, col0:], lhsT=KT[:, h, j, :],
                            rhs=QT[:, h, t0 + col0 // 128:t0 + 4, :],
                            start=True, stop=True)
                        ex = expp.tile([128, 512], BF16, tag="ex",
                                       name=f"ex{b}_{h}_{c}_{j}")
                        last_exp = nc.scalar.activation(
                            out=ex[:, col0:], in_=s_ps[:, col0:],
                            func=Exp, scale=SCALE)
                        if j - 4 * c >= 0:      # diagonal block: causal mask
                            nc.vector.tensor_mul(
                                ex[:, col0:col0 + 128], ex[:, col0:col0 + 128],
                                tri_sb[:])
                        if j == 0:
                            nc.vector.tensor_copy(out=exs[:], in_=ex[:])
                        else:
                            nc.vector.tensor_add(exs[:, col0:], exs[:, col0:],
                                                 ex[:, col0:])
                        nc.tensor.matmul(out=o_ps[:, col0:],
                                         lhsT=Vsb[:, j, h * 128:(h + 1) * 128],
                                         rhs=ex[:, col0:],
                                         start=(j == 0), stop=(j == njt - 1))
                    cs_ps = scps.tile([1, 512], F32, tag="sc",
                                      name=f"cs{b}_{h}_{c}")
                    nc.tensor.matmul(out=cs_ps[:], lhsT=ones_sb[:], rhs=exs[:],
                                     start=True, stop=True)
                    rcp = normp.tile([1, 512], F32, tag="rcp",
                                     name=f"rcp{b}_{h}_{c}")
                    nc.vector.reciprocal_approx_fast(out=rcp[:], in_=cs_ps[:])
                    bc = normp.tile([128, 512], F32, tag="bc",
                                    name=f"bc{b}_{h}_{c}")
                    nc.gpsimd.partition_broadcast(bc[:], rcp[:])
                    att = attp.tile([128, 512], BF16, tag="att",
                                    name=f"att{b}_{h}_{c}")
                    nc.vector.tensor_mul(att[:], o_ps[:], bc[:])
                    # one DMA: att [o=128, (jj r)] -> send slices [jj, o, r]
                    if b == B - 1:
                        dst = sendh[h][c * 4:(c + 1) * 4, :, :] \
                            .rearrange("a o r -> o a r")
                    else:
                        dst = send[b][c * 4:(c + 1) * 4,
                                      h * 128:(h + 1) * 128, :] \
                            .rearrange("a o r -> o a r")
                    nc.sync.dma_start(out=dst, in_=att[:])
                return last_exp

            def emit_wo(b, order_after=None):
                # rT in halves so the first wo matmuls start before the whole
                # receive buffer has landed. order_after keeps these loads
                # BEHIND earlier scalar-queue work: they wait on the AllToAll
                # and would otherwise head-of-line-block the exp activations.
                rT = rtp.tile([128, 16, 128], BF16, tag="rt", name=f"rt{b}")
                rv = recv[b][:].rearrange("c (hh p) r -> p (c hh) r", hh=2)
                d1 = nc.scalar.dma_start(out=rT[:, 0:8, :], in_=rv[:, 0:8, :])
                d2 = nc.scalar.dma_start(out=rT[:, 8:16, :], in_=rv[:, 8:16, :])
                if order_after is not None:
                    for dd in (d1, d2):
                        add_dep_helper(dd.ins, order_after.ins, sync=False,
                                       reason="rT load after scalar-queue work")
                for dc in range(4):
                    f_ps = bigps.tile([128, 512], F32, tag="big",
                                      name=f"fps{b}_{dc}")
                    for m in range(16):
                        nc.tensor.matmul(
                            out=f_ps[:], lhsT=rT[:, m, :],
                            rhs=wo_sb[:, m, dc * 512:(dc + 1) * 512],
                            start=(m == 0), stop=(m == 15))
                    fo = fop.tile([128, 512], F32, tag="fo", name=f"fo{b}_{dc}")
                    nc.scalar.activation(out=fo[:], in_=f_ps[:], func=Copy)
                    nc.sync.dma_start(
                        out=out.ap()[b, :, dc * 512:(dc + 1) * 512], in_=fo[:])

            def emit_wo_tail(b, order_after=None):
                rTs = []
                for h in range(HPC):
                    rT = rtp.tile([128, 8, 128], BF16, tag="rt", bufs=2,
                                  name=f"rtt{h}")
                    dd = nc.scalar.dma_start(
                        out=rT[:], in_=recvh[h][:].rearrange("c p r -> p c r"))
                    if order_after is not None:
                        add_dep_helper(dd.ins, order_after.ins, sync=False,
                                       reason="tail rT after last exp")
                    rTs.append(rT)
                # all h0 half-contractions first: they only need the first
                # AllToAll, so the PE starts ~15us before the h1 data lands
                fps = [bigps.tile([128, 512], F32, tag="big", name=f"fpt{dc}")
                       if dc < 2 else
                       pops.tile([128, 512], F32, tag="po", name=f"fpt{dc}")
                       for dc in range(4)]
                for h in range(HPC):
                    for dc in range(4):
                        for cc in range(NCORES):
                            nc.tensor.matmul(
                                out=fps[dc][:], lhsT=rTs[h][:, cc, :],
                                rhs=wo_sb[:, 2 * cc + h, dc * 512:(dc + 1) * 512],
                                start=(h == 0 and cc == 0),
                                stop=(h == 1 and cc == NCORES - 1))
                        if h == 1:
                            fo = fop.tile([128, 512], F32, tag="fo",
                                          name=f"fot{dc}")
                            nc.scalar.activation(out=fo[:], in_=fps[dc][:],
                                                 func=Copy)
                            nc.sync.dma_start(
                                out=out.ap()[b, :, dc * 512:(dc + 1) * 512],
                                in_=fo[:])

            # ---- schedule ---------------------------------------------------
            for b in range(B):
                g_last = emit_granule(b, 0)
                if b == 1:
                    # wo weights: gated behind the batch-1 x loads so they
                    # don't steal HBM bandwidth from the startup pipeline
                    for dc in range(4):
                        d = nc.gpsimd.dma_start(
                            out=wo_sb[:, :, dc * 512:(dc + 1) * 512],
                            in_=wo_v[:, :, dc * 512:(dc + 1) * 512])
                        add_dep_helper(d.ins, xg_dmas[2][-1].ins, sync=True,
                                       reason="wo load after startup loads")
                if b == B - 1:
                    # last batch: c=0 chunks only need granule (b,0), so they
                    # run early and the tail AllToAlls trigger sooner
                    emit_attention(b, 0, chunks=(0,))
                    emit_attention(b, 1, chunks=(0,))
                if b >= 2:
                    emit_wo(b - 2, order_after=g_last)
                emit_granule(b, 1)
                if b == B - 1:
                    emit_attention(b, 0, chunks=(1,))
                else:
                    emit_attention(b, 0)
                if b == B - 1:
                    nc.gpsimd.collective_compute(
                        "AllToAll", mybir.AluOpType.bypass,
                        replica_groups=[list(range(NCORES))],
                        ins=[sendh[0].opt()], outs=[recvh[0].opt()])
                    last_exp = emit_attention(b, 1, chunks=(1,))
                else:
                    last_exp = emit_attention(b, 1)
                if b == B - 1:
                    nc.gpsimd.collective_compute(
                        "AllToAll", mybir.AluOpType.bypass,
                        replica_groups=[list(range(NCORES))],
                        ins=[sendh[1].opt()], outs=[recvh[1].opt()])
                else:
                    nc.gpsimd.collective_compute(
                        "AllToAll", mybir.AluOpType.bypass,
                        replica_groups=[list(range(NCORES))],
                        ins=[send[b].opt()], outs=[recv[b].opt()])
            # wo for batch B-2 lands here: its 17us of ready PE work covers
            # the last batch's AllToAll latency before the tail contraction
            emit_wo(B - 2, order_after=last_exp)
            emit_wo_tail(B - 1, order_after=last_exp)

    nc.compile()
    return nc


def _prep_inputs(x, freqs, wq, wk, wv, wo):
    x = np.asarray(x, np.float32)
    freqs = np.asarray(freqs, np.float32)
    wq = np.asarray(wq, np.float32)
    wk = np.asarray(wk, np.float32)
    wv = np.asarray(wv, np.float32)
    wo = np.asarray(wo, np.float32)

    xt = np.ascontiguousarray(x.reshape(R, D).T).astype(NPBF16)
    wot = np.ascontiguousarray(wo.T).astype(NPBF16)

    cos = np.cos(freqs).T    # [64, S]
    sin = np.sin(freqs).T
    ctab = np.concatenate([cos, cos], axis=0).astype(NPBF16)   # [128, S]
    stab = np.concatenate([sin, sin], axis=0).astype(NPBF16)

    tri = np.tril(np.ones((128, 128), np.float32)).T.copy()  # tri[p,f]=1 if p<=f
    tri = tri.astype(NPBF16)

    in_maps = []
    for core in range(NCORES):
        cols = []
        for hh in range(HPC):
            head = core * HPC + hh
            rows = np.arange(head * HD, (head + 1) * HD)
            cols.append(np.concatenate([rows[0::2], rows[1::2]]))
        cols = np.concatenate(cols)
        vcols = np.arange(core * OL, (core + 1) * OL)
        wqk_host = np.concatenate([wq[cols, :].T, wk[cols, :].T], axis=1)
        in_maps.append({
            "xt": xt,
            "wqkt": np.ascontiguousarray(wqk_host).astype(NPBF16),
            "wvt": np.ascontiguousarray(wv[vcols, :].T).astype(NPBF16),
            "wot": wot,
            "ctab": ctab,
            "stab": stab,
            "tri": tri,
        })
    return in_maps


def kernel(x, freqs, mask, wq, wk, wv, wo, start_pos, _trace=False):
    # mask is the standard causal mask (applied structurally on-device);
    # start_pos is 0 for this problem shape.
    if "nc" not in _CACHED:
        _CACHED["nc"] = _build()
    nc = _CACHED["nc"]
    in_maps = _prep_inputs(x, freqs, wq, wk, wv, wo)
    # warmup execution settles PJRT dispatch, NRT comm init, core-start skew
    if os.environ.get("ATTN_TP_WARMUP", "1") == "1" and "warm" not in _CACHED:
        run_bass_kernel_spmd(nc, in_maps, core_ids=list(range(NCORES)), trace=False)
        _CACHED["warm"] = True
    res = run_bass_kernel_spmd(nc, in_maps, core_ids=list(range(NCORES)),
                               trace=_trace)
    kernel.last_results = res
    # res[j]["out"]: [B, 128, D] = rows j*128..(j+1)*128 of each batch
    parts = np.stack([res.results[j]["out"] for j in range(NCORES)], axis=1)
    return np.ascontiguousarray(parts.reshape(B, S, D)).astype(np.float32)


# revision 55
# speedup vs baseline: 1.0357x; 1.0258x over previous
"""Distributed Trainium2 (Bass/Tile) kernel for nn_Attention_10771777978397.

Strategy v2 (tensor-parallel over heads, 8 NeuronCores):
  - Each core computes Q/K projections for its 2 heads DIRECTLY in transposed
    layout [hd, r] via weight-stationary matmuls (lhsT = weight block, rhs = x),
    eliminating all DMA transposes. RoPE is applied in the transposed layout
    with cross-partition-half vector ops (weights column-permuted [evens, odds]
    per head so the pair partner sits 64 partitions away).
  - V is produced row-major [r, hd] (x-stationary) for the AV matmul.
  - Causal attention per (batch, head) in transposed-softmax layout produces
    attnT [o=128, sq] tiles; softmax denominators come from a vector-accumulated
    exp-sum and a single ones-matmul per 512-chunk (instead of one per k-tile).
  - One AllToAll PER BATCH redistributes attnT so core j holds all 16 heads for
    rows j*128..(j+1)*128 of that batch; the wo projection for those rows runs
    overlapped with the next batch's QKV compute. Host concatenates.

All matmuls bf16 with f32 PSUM accumulation. Softmax uses exp without
max-subtraction (logits bounded by construction; masked logits never computed;
causal boundary handled by a multiplicative lower-triangular mask on diagonal
128x128 blocks). Normalization is deferred to the attnT tiles.
"""

import math
import os

import numpy as np
import ml_dtypes

import concourse.bass as bass
import concourse.tile as tile
from concourse.tile import add_dep_helper
from concourse import bacc, mybir
from concourse.bass_utils import run_bass_kernel_spmd

# problem shape (hardcoded per harness contract)
B, S, D, H = 4, 1024, 2048, 16
HD = D // H          # 128
NCORES = 8
HPC = H // NCORES    # 2 heads per core
OL = HPC * HD        # 256 local o-dim
R = B * S            # 4096 rows
SCALE = 1.0 / math.sqrt(HD)
NWARM = 52           # PE warm-up matmuls (HAM clock-gate bridge)

BF16 = mybir.dt.bfloat16
F32 = mybir.dt.float32
NPBF16 = ml_dtypes.bfloat16
Copy = mybir.ActivationFunctionType.Copy
Exp = mybir.ActivationFunctionType.Exp

_CACHED = {}


def _build():
    nc = bacc.Bacc("TRN2", target_bir_lowering=False, debug=False,
                   num_devices=NCORES, name="attn_tp2")

    xt = nc.declare_dram_parameter("xt", [D, R], BF16, isOutput=False)
    wqkt = nc.declare_dram_parameter("wqkt", [D, 2 * OL], BF16, isOutput=False)
    wvt = nc.declare_dram_parameter("wvt", [D, OL], BF16, isOutput=False)
    wot = nc.declare_dram_parameter("wot", [D, D], BF16, isOutput=False)
    ctab = nc.declare_dram_parameter("ctab", [128, S], BF16, isOutput=False)
    stab = nc.declare_dram_parameter("stab", [128, S], BF16, isOutput=False)
    tri = nc.declare_dram_parameter("tri", [128, 128], BF16, isOutput=False)
    out = nc.declare_dram_parameter("out", [B, 128, D], F32, isOutput=True)

    xt_v = xt.ap().rearrange("(k p) r -> p k r", p=128)      # [128,16,R]
    wqk_v = wqkt.ap().rearrange("(k p) o -> p k o", p=128)   # [128,16,512]
    wv_v = wvt.ap().rearrange("(k p) o -> p k o", p=128)     # [128,16,256]
    wo_v = wot.ap().rearrange("(k p) o -> p k o", p=128)     # [128,16,D]

    with tile.TileContext(nc) as tc:
        with (
            tc.tile_pool(name="persist", bufs=1) as persist,
            tc.tile_pool(name="xtp", bufs=2) as xtp,
            tc.tile_pool(name="qktp", bufs=2) as qktp,
            tc.tile_pool(name="ropep", bufs=2) as ropep,
            tc.tile_pool(name="expp", bufs=4) as expp,
            tc.tile_pool(name="attp", bufs=3) as attp,
            tc.tile_pool(name="normp", bufs=2) as normp,
            tc.tile_pool(name="rtp", bufs=2) as rtp,
            tc.tile_pool(name="fop", bufs=2) as fop,
            tc.tile_pool(name="bigps", bufs=3, space="PSUM") as bigps,
            tc.tile_pool(name="scps", bufs=2, space="PSUM") as scps,
            tc.tile_pool(name="pops", bufs=2, space="PSUM") as pops,
            tc.tile_pool(name="t2ps", bufs=1, space="PSUM") as t2ps,
            tc.tile_pool(name="dram", bufs=1, space="DRAM") as dram,
        ):
            # ---- persistent SBUF loads --------------------------------------
            # wqk o-group-major (host pre-ordered by emission order of the
            # four QK groups) so the first group's weights land in ~1.5us;
            # rope tables + wv go on the gpsimd queue to keep sync free
            wqk_sb = persist.tile([128, 4, 16, 128], BF16)
            wv_sb = persist.tile([128, 16, OL], BF16)
            wqk_g = wqkt.ap().rearrange("(k p) (g o) -> p g k o", p=128, g=4)
            for grp in (0, 2, 1, 3):   # QK-group emission order
                # first group in fine chunks so granule-0 matmuls start on
                # k=0 while the rest of the startup load is still in flight
                step = 4 if grp == 0 else 16
                for kc in range(0, 16, step):
                    nc.sync.dma_start(out=wqk_sb[:, grp, kc:kc + step, :],
                                      in_=wqk_g[:, grp, kc:kc + step, :])
            ctab_sb = persist.tile([128, S], BF16)
            stab_sb = persist.tile([128, S], BF16)
            nc.gpsimd.dma_start(out=ctab_sb[:], in_=ctab.ap())
            nc.gpsimd.dma_start(out=stab_sb[:], in_=stab.ap())
            for kc in range(0, 16, 4):
                nc.gpsimd.dma_start(out=wv_sb[:, kc:kc + 4, :],
                                    in_=wv_v[:, kc:kc + 4, :])
            tri_sb = persist.tile([128, 128], BF16)
            nc.sync.dma_start(out=tri_sb[:], in_=tri.ap())
            ones_sb = persist.tile([128, 1], BF16)
            nc.vector.memset(ones_sb[:], 1.0)

            # first x granule (scalar HWDGE queue, ahead of everything else)
            xg_tiles = {}
            xg_dmas = {}

            def prefetch_xg(g, step=4):
                xg = xtp.tile([128, 16, 512], BF16, tag="xg", name=f"xg{g}")
                dmas = []
                for kc in range(0, 16, step):
                    dmas.append(nc.scalar.dma_start(
                        out=xg[:, kc:kc + step, :],
                        in_=xt_v[:, kc:kc + step, g * 512:(g + 1) * 512]))
                xg_tiles[g] = xg
                xg_dmas[g] = dmas
                return dmas

            prefetch_xg(0, step=2)

            wo_sb = persist.tile([128, 16, D], BF16)

            # PE pre-warm: dependency-free matmuls bridge the HAM clock gate
            # until the first real matmuls are ready
            warm_sb = persist.tile([128, 512], BF16, name="warm_sb")
            nc.vector.memset(warm_sb[:], 0.0)
            for w in range(NWARM):
                w_ps = scps.tile([128, 512], F32, tag="sc", name=f"warm{w}")
                nc.tensor.matmul(out=w_ps[:], lhsT=warm_sb[:, :128],
                                 rhs=warm_sb[:], start=True, stop=True)

            send = [dram.tile([NCORES, OL, 128], BF16, name=f"send{b}",
                              tag=f"send{b}") for b in range(B - 1)]
            recv = [dram.tile([NCORES, OL, 128], BF16, name=f"recv{b}",
                              tag=f"recv{b}") for b in range(B - 1)]
            # last batch: per-head buffers so its two AllToAlls pipeline
            sendh = [dram.tile([NCORES, 128, 128], BF16, name=f"sendh{h}",
                               tag=f"sendh{h}") for h in range(HPC)]
            recvh = [dram.tile([NCORES, 128, 128], BF16, name=f"recvh{h}",
                               tag=f"recvh{h}") for h in range(HPC)]
            qkt_tiles = {}
            vsb_tiles = {}

            # ---- phase blocks ----------------------------------------------
            def emit_granule(b, gi):
                g = 2 * b + gi
                if g + 1 < 2 * B:
                    prefetch_xg(g + 1)
                xg = xg_tiles.pop(g)
                soff = gi * 512
                if gi == 0:
                    QT = qktp.tile([128, HPC, 8, 128], BF16, tag="qt",
                                   name=f"qt{b}")
                    KT = qktp.tile([128, HPC, 8, 128], BF16, tag="kt",
                                   name=f"kt{b}")
                    Vsb = qktp.tile([128, 8, OL], BF16, tag="v", name=f"v{b}")
                    qkt_tiles[b] = (QT, KT)
                    vsb_tiles[b] = Vsb
                QT, KT = qkt_tiles[b]
                Vsb = vsb_tiles[b]

                def qk_group(h, qk):
                    ps = bigps.tile([128, 512], F32, tag="big",
                                    name=f"qkps{g}_{h}_{qk}")
                    grp = qk * 2 + h
                    for k in range(16):
                        nc.tensor.matmul(out=ps[:], lhsT=wqk_sb[:, grp, k, :],
                                         rhs=xg[:, k, :], start=(k == 0),
                                         stop=(k == 15))
                    # t2 lives in PSUM: a TensorTensor op may mix partition
                    # bases only when one input is PSUM (SB+SB must match)
                    t1 = ropep.tile([128, 512], BF16, tag="t1", name=f"t1_{g}{h}{qk}")
                    t2 = t2ps.tile([128, 512], F32, tag="t2", name=f"t2_{g}{h}{qk}")
                    nc.vector.tensor_mul(t1[:], ps[:], ctab_sb[:, soff:soff + 512])
                    nc.vector.tensor_mul(t2[:], ps[:], stab_sb[:, soff:soff + 512])
                    dst = QT if qk == 0 else KT
                    lo = dst[0:64, h, gi * 4:gi * 4 + 4, :]
                    hi = dst[64:128, h, gi * 4:gi * 4 + 4, :]
                    nc.vector.tensor_sub(lo, t1[0:64, :], t2[64:128, :])
                    nc.vector.tensor_add(hi, t2[0:64, :], t1[64:128, :])

                last_scalar = [None]

                def v_group(pair):
                    vp = bigps.tile([128, 512], F32, tag="big",
                                    name=f"vps{g}_{pair}")
                    for u2 in range(2):
                        u = pair * 2 + u2
                        for k in range(16):
                            nc.tensor.matmul(
                                out=vp[:, u2 * OL:(u2 + 1) * OL],
                                lhsT=xg[:, k, u * 128:(u + 1) * 128],
                                rhs=wv_sb[:, k, :], start=(k == 0), stop=(k == 15))
                        last_scalar[0] = nc.scalar.activation(
                            out=Vsb[:, gi * 4 + u, :],
                            in_=vp[:, u2 * OL:(u2 + 1) * OL], func=Copy)

                qk_group(0, 0)
                v_group(0)
                qk_group(0, 1)
                v_group(1)
                qk_group(1, 0)
                qk_group(1, 1)
                return last_scalar[0]

            def emit_attention(b, h, chunks=(0, 1)):
                QT, KT = qkt_tiles[b]
                Vsb = vsb_tiles[b]
                last_exp = None
                for c in chunks:                # sq chunks of 512
                    o_ps = pops.tile([128, 512], F32, tag="po",
                                     name=f"po{b}_{h}_{c}")
                    exs = expp.tile([128, 512], BF16, tag="exs",
                                    name=f"exs{b}_{h}_{c}", bufs=2)
                    njt = 4 * c + 4             # sk tiles for this chunk
                    for j in range(njt):
                        col0 = max(0, (j - 4 * c) * 128)
                        t0 = 4 * c
                        s_ps = scps.tile([128, 512], F32, tag="sc",
                                         name=f"sc{b}_{h}_{c}_{j}")
                        nc.tensor.matmul(
                            out=s_ps[:, col0:], lhsT=KT[:, h, j, :],
                            rhs=QT[:, h, t0 + col0 // 128:t0 + 4, :],
                            start=True, stop=True)
                        ex = expp.tile([128, 512], BF16, tag="ex",
                                       name=f"ex{b}_{h}_{c}_{j}")
                        last_exp = nc.scalar.activation(
                            out=ex[:, col0:], in_=s_ps[:, col0:],
                            func=Exp, scale=SCALE)
                        if j - 4 * c >= 0:      # diagonal block: causal mask
                            nc.vector.tensor_mul(
                                ex[:, col0:col0 + 128], ex[:, col0:col0 + 128],
                                tri_sb[:])
                        if j == 0:
                            nc.vector.tensor_copy(out=exs[:], in_=ex[:])
                        else:
                            nc.vector.tensor_add(exs[:, col0:], exs[:, col0:],
                                                 ex[:, col0:])
                        nc.tensor.matmul(out=o_ps[:, col0:],
                                         lhsT=Vsb[:, j, h * 128:(h + 1) * 128],
                                         rhs=ex[:, col0:],
                                         start=(j == 0), stop=(j == njt - 1))
                    cs_ps = scps.tile([1, 512], F32, tag="sc",
                                      name=f"cs{b}_{h}_{c}")
                    nc.tensor.matmul(out=cs_ps[:], lhsT=ones_sb[:], rhs=exs[:],
                                     start=True, stop=True)
                    rcp = normp.tile([1, 512], F32, tag="rcp",
                                     name=f"rcp{b}_{h}_{c}")
                    nc.vector.reciprocal_approx_fast(out=rcp[:], in_=cs_ps[:])
                    bc = normp.tile([128, 512], F32, tag="bc",
                                    name=f"bc{b}_{h}_{c}")
                    nc.gpsimd.partition_broadcast(bc[:], rcp[:])
                    att = attp.tile([128, 512], BF16, tag="att",
                                    name=f"att{b}_{h}_{c}")
                    nc.vector.tensor_mul(att[:], o_ps[:], bc[:])
                    # one DMA: att [o=128, (jj r)] -> send slices [jj, o, r]
                    if b == B - 1:
                        dst = sendh[h][c * 4:(c + 1) * 4, :, :] \
                            .rearrange("a o r -> o a r")
                    else:
                        dst = send[b][c * 4:(c + 1) * 4,
                                      h * 128:(h + 1) * 128, :] \
                            .rearrange("a o r -> o a r")
                    nc.sync.dma_start(out=dst, in_=att[:])
                return last_exp

            def emit_wo(b, order_after=None):
                # rT in halves so the first wo matmuls start before the whole
                # receive buffer has landed. order_after keeps these loads
                # BEHIND earlier scalar-queue work: they wait on the AllToAll
                # and would otherwise head-of-line-block the exp activations.
                rT = rtp.tile([128, 16, 128], BF16, tag="rt", name=f"rt{b}")
                rv = recv[b][:].rearrange("c (hh p) r -> p (c hh) r", hh=2)
                d1 = nc.scalar.dma_start(out=rT[:, 0:8, :], in_=rv[:, 0:8, :])
                d2 = nc.scalar.dma_start(out=rT[:, 8:16, :], in_=rv[:, 8:16, :])
                if order_after is not None:
                    for dd in (d1, d2):
                        add_dep_helper(dd.ins, order_after.ins, sync=False,
                                       reason="rT load after scalar-queue work")
                for dc in range(4):
                    f_ps = bigps.tile([128, 512], F32, tag="big",
                                      name=f"fps{b}_{dc}")
                    for m in range(16):
                        nc.tensor.matmul(
                            out=f_ps[:], lhsT=rT[:, m, :],
                            rhs=wo_sb[:, m, dc * 512:(dc + 1) * 512],
                            start=(m == 0), stop=(m == 15))
                    fo = fop.tile([128, 512], F32, tag="fo", name=f"fo{b}_{dc}")
                    nc.scalar.activation(out=fo[:], in_=f_ps[:], func=Copy)
                    nc.sync.dma_start(
                        out=out.ap()[b, :, dc * 512:(dc + 1) * 512], in_=fo[:])

            def emit_wo_tail(b, order_after=None):
                rTs = []
                for h in range(HPC):
                    rT = rtp.tile([128, 8, 128], BF16, tag="rt", bufs=2,
                                  name=f"rtt{h}")
                    dd = nc.scalar.dma_start(
                        out=rT[:], in_=recvh[h][:].rearrange("c p r -> p c r"))
                    if order_after is not None:
                        add_dep_helper(dd.ins, order_after.ins, sync=False,
                                       reason="tail rT after last exp")
                    rTs.append(rT)
                # all h0 half-contractions first: they only need the first
                # AllToAll, so the PE starts ~15us before the h1 data lands
                fps = [bigps.tile([128, 512], F32, tag="big", name=f"fpt{dc}")
                       if dc < 2 else
                       pops.tile([128, 512], F32, tag="po", name=f"fpt{dc}")
                       for dc in range(4)]
                for h in range(HPC):
                    for dc in range(4):
                        for cc in range(NCORES):
                            nc.tensor.matmul(
                                out=fps[dc][:], lhsT=rTs[h][:, cc, :],
                                rhs=wo_sb[:, 2 * cc + h, dc * 512:(dc + 1) * 512],
                                start=(h == 0 and cc == 0),
                                stop=(h == 1 and cc == NCORES - 1))
                        if h == 1:
                            fo = fop.tile([128, 512], F32, tag="fo",
                                          name=f"fot{dc}")
                            nc.scalar.activation(out=fo[:], in_=fps[dc][:],
                                                 func=Copy)
                            nc.sync.dma_start(
                                out=out.ap()[b, :, dc * 512:(dc + 1) * 512],
                                in_=fo[:])

            # ---- schedule ---------------------------------------------------
            for b in range(B):
                g_last = emit_granule(b, 0)
                if b == 1:
                    # wo weights: gated behind the batch-1 x loads so they
                    # don't steal HBM bandwidth from the startup pipeline
                    for dc in range(4):
                        d = nc.gpsimd.dma_start(
                            out=wo_sb[:, :, dc * 512:(dc + 1) * 512],
                            in_=wo_v[:, :, dc * 512:(dc + 1) * 512])
                        add_dep_helper(d.ins, xg_dmas[2][-1].ins, sync=True,
                                       reason="wo load after startup loads")
                if b == B - 1:
                    # last batch: c=0 chunks only need granule (b,0), so they
                    # run early and the tail AllToAlls trigger sooner
                    emit_attention(b, 0, chunks=(0,))
                    emit_attention(b, 1, chunks=(0,))
                if b >= 2:
                    emit_wo(b - 2, order_after=g_last)
                emit_granule(b, 1)
                if b == B - 1:
                    emit_attention(b, 0, chunks=(1,))
                else:
                    emit_attention(b, 0)
                if b == B - 1:
                    nc.gpsimd.collective_compute(
                        "AllToAll", mybir.AluOpType.bypass,
                        replica_groups=[list(range(NCORES))],
                        ins=[sendh[0].opt()], outs=[recvh[0].opt()])
                    last_exp = emit_attention(b, 1, chunks=(1,))
                else:
                    last_exp = emit_attention(b, 1)
                if b == B - 1:
                    nc.gpsimd.collective_compute(
                        "AllToAll", mybir.AluOpType.bypass,
                        replica_groups=[list(range(NCORES))],
                        ins=[sendh[1].opt()], outs=[recvh[1].opt()])
                else:
                    nc.gpsimd.collective_compute(
                        "AllToAll", mybir.AluOpType.bypass,
                        replica_groups=[list(range(NCORES))],
                        ins=[send[b].opt()], outs=[recv[b].opt()])
            # wo for batch B-2 lands here: its 17us of ready PE work covers
            # the last batch's AllToAll latency before the tail contraction
            emit_wo(B - 2, order_after=last_exp)
            emit_wo_tail(B - 1, order_after=last_exp)

    nc.compile()
    return nc


def _prep_inputs(x, freqs, wq, wk, wv, wo):
    x = np.asarray(x, np.float32)
    freqs = np.asarray(freqs, np.float32)
    wq = np.asarray(wq, np.float32)
    wk = np.asarray(wk, np.float32)
    wv = np.asarray(wv, np.float32)
    wo = np.asarray(wo, np.float32)

    xt = np.ascontiguousarray(x.reshape(R, D).T).astype(NPBF16)
    wot = np.ascontiguousarray(wo.T).astype(NPBF16)

    cos = np.cos(freqs).T    # [64, S]
    sin = np.sin(freqs).T
    ctab = np.concatenate([cos, cos], axis=0).astype(NPBF16)   # [128, S]
    stab = np.concatenate([sin, sin], axis=0).astype(NPBF16)

    tri = np.tril(np.ones((128, 128), np.float32)).T.copy()  # tri[p,f]=1 if p<=f
    tri = tri.astype(NPBF16)

    in_maps = []
    for core in range(NCORES):
        cols = []
        for hh in range(HPC):
            head = core * HPC + hh
            rows = np.arange(head * HD, (head + 1) * HD)
            cols.append(np.concatenate([rows[0::2], rows[1::2]]))
        cols = np.concatenate(cols)
        vcols = np.arange(core * OL, (core + 1) * OL)
        wqk_host = np.concatenate([wq[cols, :].T, wk[cols, :].T], axis=1)
        in_maps.append({
            "xt": xt,
            "wqkt": np.ascontiguousarray(wqk_host).astype(NPBF16),
            "wvt": np.ascontiguousarray(wv[vcols, :].T).astype(NPBF16),
            "wot": wot,
            "ctab": ctab,
            "stab": stab,
            "tri": tri,
        })
    return in_maps


def kernel(x, freqs, mask, wq, wk, wv, wo, start_pos, _trace=False):
    # mask is the standard causal mask (applied structurally on-device);
    # start_pos is 0 for this problem shape.
    if "nc" not in _CACHED:
        _CACHED["nc"] = _build()
    nc = _CACHED["nc"]
    in_maps = _prep_inputs(x, freqs, wq, wk, wv, wo)
    # warmup execution settles PJRT dispatch, NRT comm init, core-start skew
    if os.environ.get("ATTN_TP_WARMUP", "1") == "1" and "warm" not in _CACHED:
        run_bass_kernel_spmd(nc, in_maps, core_ids=list(range(NCORES)), trace=False)
        _CACHED["warm"] = True
    res = run_bass_kernel_spmd(nc, in_maps, core_ids=list(range(NCORES)),
                               trace=_trace)
    kernel.last_results = res
    # res[j]["out"]: [B, 128, D] = rows j*128..(j+1)*128 of each batch
    parts = np.stack([res.results[j]["out"] for j in range(NCORES)], axis=1)
    return np.ascontiguousarray(parts.reshape(B, S, D)).astype(np.float32)
